# revision 49
# baseline (speedup 1.0000x reference)
"""Bass/Trainium2 kernel for attention-LSTM decoder (nn_Attention_49289044688898).

Data-parallel over batch: 512 rows -> 8 NeuronCores x 64 rows. Weights replicated.
Within a core, 64 rows split into two groups of 32; per decode step s (26 steps):

  qT  = Wh^T-chunks @ hT                          (PE, transposed form)
  arg = HprojT + qT (broadcast over t)            (DVE bf16, per k-chunk, groups interleaved)
  th  = tanh(arg)                                 (ACT; the per-step bottleneck)
  e   = ws-col-groups @ th -> pe [128,512]        (PE, 4 concurrent col-group streams)
  expe = exp(pe)                                  (ACT, no max-subtract; |e| <~ 4)
  scatter expe -> affine block-diag ablk          (2 SBUF->SBUF DMAs, contiguous 16-el runs)
  uctx = ablk @ bHc ; den = ablk @ ones           (PE, 4 col groups: ctx0/ctx1/den0/den1)
  ctx  = uctx * (1/den)                           (DVE)
  z    = ctx @ Kc + h @ R + onehot @ Ko'          (PE; R/Ko streamed at step start, Kc per
                                                   group in concurrent col groups)
  gates / c / h                                   (ACT/DVE, joint)
  hT, xT via PE transpose                         (PE + DVE copy)
  probs[:, s, :] = h @ Wgen + bgen                (PE)

The tanh/e buffers order batch columns as B' = 16*(b%2) + b//2 so the block-diag
scatter writes contiguous runs. The block-diag lives at affine address
addr(kt, b) = 16*b - 31*kt + 465 in a [128, 962] tile: only diagonal entries
(b = 2*kt + par) collide with diagonal addresses, everything else stays zero.
"""

import numpy as np
import ml_dtypes
from contextlib import ExitStack

B, T, C, H, NCC, S = 512, 64, 512, 512, 96, 26
NCORES = 8
BS = B // NCORES          # 64 batch rows per core
NG = 2                    # groups per core
GB = BS // NG             # 32 rows per group
ABW = 962                 # block-diag width: 16*31 - 31*15 + 465 = 961 max
ABOFF = 465               # offset so addresses stay >= 0 (31*15)
BF = ml_dtypes.bfloat16

_CACHE = {}


def build_bass():
    import concourse.bass as bass
    import concourse.bacc as bacc
    import concourse.tile as tile
    import concourse.mybir as mybir

    f32 = mybir.dt.float32
    bf16 = mybir.dt.bfloat16
    AF = mybir.ActivationFunctionType

    nc = bacc.Bacc("TRN2", target_bir_lowering=False)

    # ---- DRAM I/O ----
    bHT_d = nc.dram_tensor("bHT", [NG, C, T, GB], bf16, kind="ExternalInput")
    bHc_d = nc.dram_tensor("bHc", [NG, GB // 2, 128, C], bf16, kind="ExternalInput")
    wi_d = nc.dram_tensor("wi", [C, H], bf16, kind="ExternalInput")
    wh_d = nc.dram_tensor("wh", [H, H], bf16, kind="ExternalInput")
    bh_d = nc.dram_tensor("bh", [128, 4], f32, kind="ExternalInput")
    ws_d = nc.dram_tensor("ws", [128, 4, 32], bf16, kind="ExternalInput")
    kc_d = nc.dram_tensor("kc", [C, 4 * H], bf16, kind="ExternalInput")
    rr_d = nc.dram_tensor("rr", [H, 4 * H], bf16, kind="ExternalInput")
    ko_d = nc.dram_tensor("ko", [NCC, 4 * H], bf16, kind="ExternalInput")
    oh_d = nc.dram_tensor("oh", [NCC, S, BS], bf16, kind="ExternalInput")
    wg_d = nc.dram_tensor("wg", [H, NCC], bf16, kind="ExternalInput")
    bg_d = nc.dram_tensor("bg", [BS, NCC], f32, kind="ExternalInput")
    eye_d = nc.dram_tensor("eye", [64, 64], bf16, kind="ExternalInput")
    out_d = nc.dram_tensor("out", [BS, S, NCC], f32, kind="ExternalOutput")

    NCH = T * GB // 512  # 4 t-chunks of 512 cols per group

    with tile.TileContext(nc) as tc, ExitStack() as ctx:
        big = ctx.enter_context(tc.tile_pool(name="big", bufs=1))
        wpool = ctx.enter_context(tc.tile_pool(name="wpool", bufs=1))
        small = ctx.enter_context(tc.tile_pool(name="small", bufs=2))
        tiny = ctx.enter_context(tc.tile_pool(name="tiny", bufs=4))
        gates = ctx.enter_context(tc.tile_pool(name="gates", bufs=4))
        state = ctx.enter_context(tc.tile_pool(name="state", bufs=2))
        # PSUM banks: pz [64,2048] = 4 + pbig x2 = 2 + psm 1 + pqd 1 = 8
        pzg = ctx.enter_context(tc.tile_pool(name="pzg", bufs=1, space="PSUM"))
        pbig = ctx.enter_context(tc.tile_pool(name="pbig", bufs=2, space="PSUM"))
        psm = ctx.enter_context(tc.tile_pool(name="psm", bufs=1, space="PSUM"))
        pqdp = ctx.enter_context(tc.tile_pool(name="pqdp", bufs=1, space="PSUM"))

        dma = nc.sync
        import concourse.bass as _b

        # ---- small weights first (the prolog needs wi immediately) ----
        wi = wpool.tile([128, 4, H], bf16, tag="wi")
        dma.dma_start(out=wi, in_=wi_d[:].rearrange("(ch cl) h -> cl ch h", cl=128))
        wh = wpool.tile([128, 4, H], bf16, tag="wh")
        dma.dma_start(out=wh, in_=wh_d[:].rearrange("(hh hl) h -> hl hh h", hl=128))
        bh = wpool.tile([128, 4], f32, tag="bh")
        dma.dma_start(out=bh, in_=bh_d[:])
        ws = wpool.tile([128, 4, 32], bf16, tag="ws")
        dma.dma_start(out=ws, in_=ws_d[:])
        # bHT feeds the prolog: chunked per (g, n) so the first Hproj
        # matmuls start right after wi/wh plus 1/8 of it land; kc/rr and
        # bHc are not needed until step 0's LSTM/ctx and load afterwards
        bHT = [big.tile([128, 4, T * GB], bf16, tag=f"th{g}", name=f"bHT{g}")
               for g in range(NG)]
        for g in range(NG):
            for n in range(NCH):
                dma.dma_start(
                    out=bHT[g][:, :, n * 512:(n + 1) * 512],
                    in_=bHT_d[g][:, 16 * n:16 * (n + 1), :]
                    .rearrange("(ch cl) t b -> cl ch (t b)", cl=128))
        bHc = [big.tile([128, GB // 2, C], bf16, tag=f"bHc{g}", name=f"bHc{g}")
               for g in range(NG)]
        kc = wpool.tile([128, 4, 4 * H], bf16, tag="kc")
        dma.dma_start(out=kc, in_=kc_d[:].rearrange("(kh kl) n -> kl kh n", kl=128))
        rr = wpool.tile([128, 4, 4 * H], bf16, tag="rr")
        dma.dma_start(out=rr, in_=rr_d[:].rearrange("(kh kl) n -> kl kh n", kl=128))
        ko = wpool.tile([NCC, 4 * H], bf16, tag="ko")
        dma.dma_start(out=ko, in_=ko_d[:])
        oh = wpool.tile([NCC, S, BS], bf16, tag="oh")
        dma.dma_start(out=oh, in_=oh_d[:])
        wg = wpool.tile([128, 4, NCC], bf16, tag="wg")
        dma.dma_start(out=wg, in_=wg_d[:].rearrange("(hh hl) n -> hl hh n", hl=128))
        bg = wpool.tile([BS, NCC], f32, tag="bg")
        dma.dma_start(out=bg, in_=bg_d[:])
        eye = wpool.tile([64, 64], bf16, tag="eye")
        dma.dma_start(out=eye, in_=eye_d[:])
        for g in range(NG):
            dma.dma_start(out=bHc[g], in_=bHc_d[g].rearrange("k p c -> p k c"))
        ones1 = wpool.tile([128, 1], bf16, tag="ones1")
        nc.vector.memset(ones1, 1.0)

        # affine block-diag holders (zeroed once; only diagonal slots written)
        ablk = [wpool.tile([128, ABW], bf16, tag=f"ablk{g}", name=f"ablk{g}")
                for g in range(NG)]
        for g in range(NG):
            nc.vector.memset(ablk[g], 0.0)

        def ablk_lhsT(g, kt):
            ab = ablk[g][:]
            return _b.AP(tensor=ab.tensor, offset=ab.offset + ABOFF - 31 * kt,
                         ap=[ab.ap[0], [16, 32]])

        # initial state (joint across groups)
        hTj = [state.tile([128, 4, BS], bf16, tag="hT", name="hT0")]
        nc.vector.memset(hTj[0], 0.0)
        c_stj = [state.tile([BS, H], f32, tag="c", name="c0")]
        nc.vector.memset(c_stj[0], 0.0)

        # ---- prolog: HprojT[g] = (batch_H @ Wi)^T + bh ----
        hprojT = [big.tile([128, 4, T * GB], bf16, tag=f"hp{g}", name=f"hp{g}")
                  for g in range(NG)]
        for g in range(NG):
            for m in range(4):
                for n in range(NCH):
                    ps = pbig.tile([128, 512], f32, tag="pbig")
                    for k in range(4):
                        nc.tensor.matmul(
                            ps,
                            wi[:, k, m * 128:(m + 1) * 128],
                            bHT[g][:, k, n * 512:(n + 1) * 512],
                            start=(k == 0), stop=(k == 3),
                        )
                    nc.scalar.activation(
                        out=hprojT[g][:, m, n * 512:(n + 1) * 512], in_=ps,
                        func=AF.Identity, bias=bh[:, m:m + 1], scale=1.0,
                    )

        def bcast_q(qTb, g, k):
            # qTb[:, k, g, :] = [128, 32] in B' order -> [128, T(bcast), 32]
            ap2 = qTb[:, k, g, 0, :]
            return _b.AP(tensor=ap2.tensor, offset=ap2.offset,
                         ap=[ap2.ap[0], [0, T], [1, 32]])

        # z columns reordered to (i, f, o, g) so the three sigmoid gates are
        # contiguous and one ACT instruction covers them
        gate_sl = {"i": 0, "f": 1, "o": 2, "g": 3}
        GORDER = ("i", "f", "o", "g")

        # pqd bank layout (one PSUM bank, f32 cols): [0,256) = qT (4 m-chunks
        # of 64 b-cols), [256,258) = softmax denominators, [258,354) = wgen
        def phase_qg(s, g, pqd, qTb):
            # q for group g of the NEXT step, from this step's hT_g
            for m in range(4):
                for k in range(4):
                    nc.tensor.matmul(
                        pqd[:, m * 64 + 32 * g:m * 64 + 32 * g + 32],
                        wh[:, k, m * 128:(m + 1) * 128],
                        hTj[0][:, k, 32 * g:32 * g + 32],
                        start=(k == 0), stop=(k == 3))
            # SBUF copy with columns permuted to B' = 16*(b%2) + b//2 order
            # so the broadcast-add APs stay stride-1 packed (DVE 2x mode)
            pqa = pqd[:]
            src = _b.AP(tensor=pqa.tensor, offset=pqa.offset + 32 * g,
                        ap=[pqa.ap[0], [64, 4], [1, 2], [2, 16]])
            nc.vector.tensor_copy(qTb[:, :, g, :, :], src)

        def phase_zh(s, pz):
            for gname in GORDER:
                zsl = slice(gate_sl[gname] * 512, (gate_sl[gname] + 1) * 512)
                for k in range(4):
                    nc.tensor.matmul(pz[:, zsl], hTj[0][:, k, :], rr[:, k, zsl],
                                     start=(k == 0), stop=False)
                nc.tensor.matmul(pz[:, zsl], oh[:, s, :], ko[:, zsl],
                                 start=False, stop=False)

        def att_front(s, g, u, qTb, th):
            # k-pair (2u, 2u+1) in one add + one tanh: halves ACT/DVE
            # instruction overhead on the attention cadence path
            ksl = slice(2 * u, 2 * u + 2)
            thv = th[:, ksl, :].rearrange("p k (t b) -> p k t b", t=T)
            hpv = hprojT[g][:, ksl, :].rearrange("p k (t b) -> p k t b", t=T)
            qa = qTb[:, 2 * u, g, 0, :]
            qv = _b.AP(tensor=qa.tensor, offset=qa.offset,
                       ap=[qa.ap[0], [64, 2], [0, T], [1, 32]])
            nc.vector.tensor_add(thv, hpv, qv)
            nc.scalar.activation(out=th[:, ksl, :], in_=th[:, ksl, :],
                                 func=AF.Tanh)

        def att_emm(s, g, k, th, pe):
            for j in range(NCH):
                bp = 32 * j
                nc.tensor.matmul(pe[bp:bp + 32, :], ws[:, k, :],
                                 th[:, k, j * 512:(j + 1) * 512],
                                 start=(k == 0), stop=(k == 3),
                                 tile_position=(0, bp))

        def att_tail(s, g, pe):
            # exp + contiguous scatter into the affine block-diag
            expe = small.tile([128, 512], bf16, tag=f"expe{g}", bufs=1,
                              name=f"expe{g}_{s}")
            nc.scalar.activation(out=expe, in_=pe, func=AF.Exp)
            ea = expe[:]
            ab = ablk[g][:]
            ps_e = ea.ap[0][0]
            ps_a = ab.ap[0][0]
            for par in (0, 1):
                src = _b.AP(tensor=ea.tensor, offset=ea.offset + 16 * par,
                            ap=[[32 * ps_e, 4], [32, 16], [1, 16]])
                dst = _b.AP(tensor=ab.tensor,
                            offset=ab.offset + 64 * par * ps_a + ABOFF + 16 * par,
                            ap=[[ps_a, 64], [1, 16]])
                dma.dma_start(out=dst, in_=src)

        def phase_ctx_g(s, g, pqd, pctx, rcp):
            # ctx at col group 32g, denominator at col group 64+32g
            for kt in range(GB // 2):
                lt = ablk_lhsT(g, kt)
                nc.tensor.matmul(pctx[32 * g:32 * g + 32, :], lt,
                                 bHc[g][:, kt, :],
                                 start=(kt == 0), stop=(kt == GB // 2 - 1))
                nc.tensor.matmul(pqd[64 + 32 * g:96 + 32 * g, 256 + g:257 + g],
                                 lt, ones1,
                                 start=(kt == 0), stop=(kt == GB // 2 - 1),
                                 tile_position=(0, 64 + 32 * g))
            gsl = slice(64 + 32 * g, 96 + 32 * g)
            nc.vector.reciprocal(rcp[gsl, g:g + 1], pqd[gsl, 256 + g:257 + g])

        def phase_scale_x(s, g, pctx, rcp, xTc):
            gsl = slice(g * GB, (g + 1) * GB)
            ctx_g = small.tile([GB, C], bf16, tag=f"ctxg{g}", bufs=2,
                               name=f"ctx{g}_{s}")
            nc.vector.tensor_scalar_mul(ctx_g,
                                        pctx[32 * g:32 * g + 32, :],
                                        rcp[64 + 32 * g:96 + 32 * g, g:g + 1])
            xTp = psm.tile([128, 4, GB], bf16, tag="psm", name=f"xTp{g}_{s}")
            for k in range(4):
                nc.tensor.transpose(xTp[:, k, :],
                                    ctx_g[:, k * 128:(k + 1) * 128],
                                    eye[0:GB, 0:GB])
            nc.vector.tensor_copy(xTc[:, :, gsl], xTp)

        def phase_kc(s, g, xTc, pz):
            gsl = slice(g * GB, (g + 1) * GB)
            for gname in GORDER:
                zsl = slice(gate_sl[gname] * 512, (gate_sl[gname] + 1) * 512)
                for k in range(4):
                    nc.tensor.matmul(pz[gsl, zsl],
                                     xTc[:, k, gsl], kc[:, k, zsl],
                                     start=False, stop=(k == 3))

        def phase_gates(s, g, pz, c_new, h_bf):
            # per-gate ACTs so the c-chain pipelines behind f/i/g
            # (all tensor_tensor operands sliced to the same partition base)
            gsl = slice(g * GB, (g + 1) * GB)
            sig = {}
            for gname in ("f", "i", "g", "o"):
                zsl = slice(gate_sl[gname] * 512, (gate_sl[gname] + 1) * 512)
                g_sb = gates.tile([BS, 512], f32, tag=f"gate{g}", bufs=3)
                if gname == "g":
                    nc.scalar.activation(out=g_sb[gsl, :], in_=pz[gsl, zsl],
                                         func=AF.Tanh)
                else:
                    nc.scalar.activation(out=g_sb[gsl, :], in_=pz[gsl, zsl],
                                         func=AF.Tanh, scale=0.5)
                    nc.vector.tensor_scalar(out=g_sb[gsl, :], in0=g_sb[gsl, :],
                                            scalar1=0.5, scalar2=0.5,
                                            op0=mybir.AluOpType.mult,
                                            op1=mybir.AluOpType.add)
                sig[gname] = g_sb
                if gname == "i":
                    t1 = gates.tile([BS, H], f32, tag=f"tmp{g}", bufs=2,
                                    name=f"t1_{g}_{s}")
                    sig["t1"] = t1
                    nc.vector.tensor_mul(t1[gsl, :], sig["f"][gsl, :],
                                         c_stj[0][gsl, :])
                elif gname == "g":
                    t2 = gates.tile([BS, H], f32, tag=f"tmp{g}", bufs=2)
                    nc.vector.tensor_mul(t2[gsl, :], sig["i"][gsl, :],
                                         sig["g"][gsl, :])
                    nc.vector.tensor_add(c_new[gsl, :], sig["t1"][gsl, :],
                                         t2[gsl, :])
            tc_sb = gates.tile([BS, H], f32, tag=f"tmp{g}", bufs=2)
            nc.scalar.activation(out=tc_sb[gsl, :], in_=c_new[gsl, :],
                                 func=AF.Tanh)
            nc.vector.tensor_mul(h_bf[gsl, :], sig["o"][gsl, :], tc_sb[gsl, :])

        def phase_hq_g(s, g, h_bf, hTn, pqd, qTb):
            # per-group hT via PE transpose + next-step q_g: group 0 runs right
            # after gates0 so next step's attention front-runs the g1 tail
            gsl = slice(g * GB, (g + 1) * GB)
            hTp = psm.tile([128, 4, GB], bf16, tag="psm", name=f"hTp{g}_{s}")
            for k in range(4):
                # eye's diagonal block at rows 32g.. matches h_bf's base
                nc.tensor.transpose(hTp[:, k, :],
                                    h_bf[gsl, k * 128:(k + 1) * 128],
                                    eye[gsl, gsl])
            nc.vector.tensor_copy(hTn[:, :, gsl], hTp)
            for m in range(4):
                for k in range(4):
                    nc.tensor.matmul(
                        pqd[:, m * 64 + 32 * g:m * 64 + 32 * g + 32],
                        wh[:, k, m * 128:(m + 1) * 128],
                        hTn[:, k, gsl],
                        start=(k == 0), stop=(k == 3))
            pqa = pqd[:]
            src = _b.AP(tensor=pqa.tensor, offset=pqa.offset + 32 * g,
                        ap=[pqa.ap[0], [64, 4], [1, 2], [2, 16]])
            nc.vector.tensor_copy(qTb[:, :, g, :, :], src)

        def phase_tail2(s, h_bf, hTn, pqd, qTb):
            phase_hq_g(s, 1, h_bf, hTn, pqd, qTb)
            hTj[0] = hTn
            for k in range(4):
                nc.tensor.matmul(pqd[0:BS, 258:258 + NCC],
                                 hTn[:, k, :], wg[:, k, :],
                                 start=(k == 0), stop=(k == 3))
            pr_sb = small.tile([BS, NCC], f32, tag="pr_sb", bufs=2, name=f"pr_{s}")
            nc.vector.tensor_add(pr_sb, pqd[0:BS, 258:258 + NCC], bg)
            dma.dma_start(out=out_d[:, s, :], in_=pr_sb)

        # software-pipelined main loop: group 0's next-step q/attention overlap
        # group 1's current-step tail via per-group q/hT/wgen splits
        qTb = small.tile([128, 4, NG, 2, 16], bf16, tag="qT", bufs=2, name="qT0")
        nc.vector.memset(qTb, 0.0)
        pz = pzg.tile([BS, 4 * 512], f32, tag="pz", name="pz_0")
        phase_zh(0, pz)
        for s in range(S):
            th = [None, None]
            th[0] = big.tile([128, 4, T * GB], bf16, tag="th0", name=f"th0_{s}")
            pe0 = pbig.tile([128, 512], f32, tag="pbig", name=f"pe0_{s}")
            for u in range(2):
                att_front(s, 0, u, qTb, th[0])
                att_emm(s, 0, 2 * u, th[0], pe0)
                att_emm(s, 0, 2 * u + 1, th[0], pe0)
            att_tail(s, 0, pe0)
            th[1] = big.tile([128, 4, T * GB], bf16, tag="th1", name=f"th1_{s}")
            for u in range(2):
                att_front(s, 1, u, qTb, th[1])
            pe1 = pbig.tile([128, 512], f32, tag="pbig", name=f"pe1_{s}")
            for k in range(4):
                att_emm(s, 1, k, th[1], pe1)
            pqd = pqdp.tile([128, 354], f32, tag="pqd", name=f"pqd_{s}")
            rcp = tiny.tile([128, 2], f32, tag="rcp", name=f"rcp_{s}")
            xTc = small.tile([128, 4, BS], bf16, tag="xTc", bufs=2, name=f"xTc_{s}")
            pctx0 = pbig.tile([128, 512], f32, tag="pbig", name=f"pctx0_{s}")
            phase_ctx_g(s, 0, pqd, pctx0, rcp)
            phase_scale_x(s, 0, pctx0, rcp, xTc)
            phase_kc(s, 0, xTc, pz)
            att_tail(s, 1, pe1)
            c_new = state.tile([BS, H], f32, tag="c", name=f"c_{s}")
            h_bf = small.tile([BS, H], bf16, tag="h_bf", bufs=1, name=f"h_bf_{s}")
            qTb_n = small.tile([128, 4, NG, 2, 16], bf16, tag="qT", bufs=2,
                               name=f"qT_{s + 1}")
            hTn = small.tile([128, 4, BS], bf16, tag="hT", bufs=2,
                             name=f"hT_{s}")
            phase_gates(s, 0, pz, c_new, h_bf)
            phase_hq_g(s, 0, h_bf, hTn, pqd, qTb_n)
            pctx1 = pbig.tile([128, 512], f32, tag="pbig", name=f"pctx1_{s}")
            phase_ctx_g(s, 1, pqd, pctx1, rcp)
            phase_scale_x(s, 1, pctx1, rcp, xTc)
            phase_kc(s, 1, xTc, pz)
            phase_gates(s, 1, pz, c_new, h_bf)
            c_stj[0] = c_new
            phase_tail2(s, h_bf, hTn, pqd, qTb_n)
            qTb = qTb_n
            if s + 1 < S:
                pz = pzg.tile([BS, 4 * 512], f32, tag="pz", name=f"pz_{s + 1}")
                phase_zh(s + 1, pz)

    nc.finalize()
    return nc


def _prep_core(inputs, i):
    bsl = slice(i * BS, (i + 1) * BS)
    bh_i = np.asarray(inputs["batch_H"][bsl], np.float32)          # [64, 64, 512]
    text_i = np.asarray(inputs["text"][bsl])                       # [64, 26]
    bh_g = bh_i.reshape(NG, GB, T, C)
    m = {}
    # bHT columns in B' = 16*(b%2) + b//2 order (for the contiguous scatter)
    perm = np.array([2 * (bp % 16) + bp // 16 for bp in range(GB)])
    bht = np.ascontiguousarray(bh_g.transpose(0, 3, 2, 1))         # [g, c, t, b]
    m["bHT"] = np.ascontiguousarray(bht[:, :, :, perm]).astype(BF)
    m["bHc"] = np.ascontiguousarray(bh_g.reshape(NG, GB // 2, 128, C)).astype(BF)
    m["wi"] = np.asarray(inputs["Wi"], np.float32).astype(BF)
    m["wh"] = np.asarray(inputs["Wh"], np.float32).astype(BF)
    m["bh"] = np.ascontiguousarray(
        np.asarray(inputs["bh"], np.float32).reshape(4, 128).T)
    wsr = np.ascontiguousarray(
        np.asarray(inputs["Ws"], np.float32)[:, 0].reshape(4, 128).T).astype(BF)
    m["ws"] = np.repeat(wsr[:, :, None], 32, axis=2)
    lk = np.asarray(inputs["lstm_kernel"], np.float32)
    lb = np.asarray(inputs["lstm_bias"], np.float32)
    # z blocks reordered (i, f, g, o) -> (i, f, o, g)
    zperm = np.concatenate([np.arange(0, 1024), np.arange(1536, 2048),
                            np.arange(1024, 1536)])
    m["kc"] = lk[:C][:, zperm].astype(BF)
    m["ko"] = (lk[C:] + lb[None, :])[:, zperm].astype(BF)
    m["rr"] = np.asarray(inputs["lstm_rec"], np.float32)[:, zperm].astype(BF)
    m["oh"] = (np.arange(NCC)[:, None, None] == text_i.T[None, :, :]).astype(BF)
    m["wg"] = np.asarray(inputs["Wgen"], np.float32).astype(BF)
    m["bg"] = np.tile(np.asarray(inputs["bgen"], np.float32)[None, :], (BS, 1))
    m["eye"] = np.eye(64, dtype=np.float32).astype(BF)
    return m


def kernel(_trace=False, **inputs):
    from concourse import bass_utils
    if "nc" not in _CACHE:
        _CACHE["nc"] = build_bass()
    nc = _CACHE["nc"]
    in_maps = [_prep_core(inputs, i) for i in range(NCORES)]
    res = bass_utils.run_bass_kernel_spmd(nc, in_maps, list(range(NCORES)),
                                          trace=_trace)
    _CACHE["last_result"] = res
    out = np.concatenate([r["out"] for r in res.results], axis=0)
    return out.astype(np.float32)


# revision 51
# speedup vs baseline: 1.0192x; 1.0192x over previous
"""Bass/Trainium2 kernel for attention-LSTM decoder (nn_Attention_49289044688898).

Data-parallel over batch: 512 rows -> 8 NeuronCores x 64 rows. Weights replicated.
Within a core, 64 rows split into two groups of 32; per decode step s (26 steps):

  qT  = Wh^T-chunks @ hT                          (PE, transposed form)
  arg = HprojT + qT (broadcast over t)            (DVE bf16, per k-chunk, groups interleaved)
  th  = tanh(arg)                                 (ACT; the per-step bottleneck)
  e   = ws-col-groups @ th -> pe [128,512]        (PE, 4 concurrent col-group streams)
  expe = exp(pe)                                  (ACT, no max-subtract; |e| <~ 4)
  scatter expe -> affine block-diag ablk          (2 SBUF->SBUF DMAs, contiguous 16-el runs)
  uctx = ablk @ bHc ; den = ablk @ ones           (PE, 4 col groups: ctx0/ctx1/den0/den1)
  ctx  = uctx * (1/den)                           (DVE)
  z    = ctx @ Kc + h @ R + onehot @ Ko'          (PE; R/Ko streamed at step start, Kc per
                                                   group in concurrent col groups)
  gates / c / h                                   (ACT/DVE, joint)
  hT, xT via PE transpose                         (PE + DVE copy)
  probs[:, s, :] = h @ Wgen + bgen                (PE)

The tanh/e buffers order batch columns as B' = 16*(b%2) + b//2 so the block-diag
scatter writes contiguous runs. The block-diag lives at affine address
addr(kt, b) = 16*b - 31*kt + 465 in a [128, 962] tile: only diagonal entries
(b = 2*kt + par) collide with diagonal addresses, everything else stays zero.
"""

import numpy as np
import ml_dtypes
from contextlib import ExitStack

B, T, C, H, NCC, S = 512, 64, 512, 512, 96, 26
NCORES = 8
BS = B // NCORES          # 64 batch rows per core
NG = 2                    # groups per core
GB = BS // NG             # 32 rows per group
ABW = 962                 # block-diag width: 16*31 - 31*15 + 465 = 961 max
ABOFF = 465               # offset so addresses stay >= 0 (31*15)
BF = ml_dtypes.bfloat16

_CACHE = {}


def build_bass():
    import concourse.bass as bass
    import concourse.bacc as bacc
    import concourse.tile as tile
    import concourse.mybir as mybir

    f32 = mybir.dt.float32
    bf16 = mybir.dt.bfloat16
    AF = mybir.ActivationFunctionType

    nc = bacc.Bacc("TRN2", target_bir_lowering=False)

    # ---- DRAM I/O ----
    bHT_d = nc.dram_tensor("bHT", [NG, C, T, GB], bf16, kind="ExternalInput")
    bHc_d = nc.dram_tensor("bHc", [NG, GB // 2, 128, C], bf16, kind="ExternalInput")
    wi_d = nc.dram_tensor("wi", [C, H], bf16, kind="ExternalInput")
    wh_d = nc.dram_tensor("wh", [H, H], bf16, kind="ExternalInput")
    bh_d = nc.dram_tensor("bh", [128, 4], f32, kind="ExternalInput")
    ws_d = nc.dram_tensor("ws", [128, 4, 32], bf16, kind="ExternalInput")
    kc_d = nc.dram_tensor("kc", [C, 4 * H], bf16, kind="ExternalInput")
    rr_d = nc.dram_tensor("rr", [H, 4 * H], bf16, kind="ExternalInput")
    ko_d = nc.dram_tensor("ko", [NCC, 4 * H], bf16, kind="ExternalInput")
    oh_d = nc.dram_tensor("oh", [NCC, S, BS], bf16, kind="ExternalInput")
    wg_d = nc.dram_tensor("wg", [H, NCC], bf16, kind="ExternalInput")
    bg_d = nc.dram_tensor("bg", [BS, NCC], f32, kind="ExternalInput")
    eye_d = nc.dram_tensor("eye", [64, 64], bf16, kind="ExternalInput")
    out_d = nc.dram_tensor("out", [BS, S, NCC], f32, kind="ExternalOutput")

    NCH = T * GB // 512  # 4 t-chunks of 512 cols per group

    with tile.TileContext(nc) as tc, ExitStack() as ctx:
        big = ctx.enter_context(tc.tile_pool(name="big", bufs=1))
        wpool = ctx.enter_context(tc.tile_pool(name="wpool", bufs=1))
        small = ctx.enter_context(tc.tile_pool(name="small", bufs=2))
        tiny = ctx.enter_context(tc.tile_pool(name="tiny", bufs=4))
        gates = ctx.enter_context(tc.tile_pool(name="gates", bufs=4))
        state = ctx.enter_context(tc.tile_pool(name="state", bufs=2))
        # PSUM banks: pz [64,2048] = 4 + pbig x2 = 2 + psm 1 + pqd 1 = 8
        pzg = ctx.enter_context(tc.tile_pool(name="pzg", bufs=1, space="PSUM"))
        pbig = ctx.enter_context(tc.tile_pool(name="pbig", bufs=2, space="PSUM"))
        psm = ctx.enter_context(tc.tile_pool(name="psm", bufs=1, space="PSUM"))
        pqdp = ctx.enter_context(tc.tile_pool(name="pqdp", bufs=1, space="PSUM"))

        dma = nc.sync
        import concourse.bass as _b

        # ---- small weights first (the prolog needs wi immediately) ----
        wi = wpool.tile([128, 4, H], bf16, tag="wi")
        dma.dma_start(out=wi, in_=wi_d[:].rearrange("(ch cl) h -> cl ch h", cl=128))
        wh = wpool.tile([128, 4, H], bf16, tag="wh")
        dma.dma_start(out=wh, in_=wh_d[:].rearrange("(hh hl) h -> hl hh h", hl=128))
        bh = wpool.tile([128, 4], f32, tag="bh")
        dma.dma_start(out=bh, in_=bh_d[:])
        ws = wpool.tile([128, 4, 32], bf16, tag="ws")
        dma.dma_start(out=ws, in_=ws_d[:])
        # bHT feeds the prolog: chunked per (g, n) so the first Hproj
        # matmuls start right after wi/wh plus 1/8 of it land; kc/rr and
        # bHc are not needed until step 0's LSTM/ctx and load afterwards
        bHT = [big.tile([128, 4, T * GB], bf16, tag=f"th{g}", name=f"bHT{g}")
               for g in range(NG)]
        for g in range(NG):
            for n in range(NCH):
                dma.dma_start(
                    out=bHT[g][:, :, n * 512:(n + 1) * 512],
                    in_=bHT_d[g][:, 16 * n:16 * (n + 1), :]
                    .rearrange("(ch cl) t b -> cl ch (t b)", cl=128))
        bHc = [big.tile([128, GB // 2, C], bf16, tag=f"bHc{g}", name=f"bHc{g}")
               for g in range(NG)]
        kc = wpool.tile([128, 4, 4 * H], bf16, tag="kc")
        dma.dma_start(out=kc, in_=kc_d[:].rearrange("(kh kl) n -> kl kh n", kl=128))
        rr = wpool.tile([128, 4, 4 * H], bf16, tag="rr")
        dma.dma_start(out=rr, in_=rr_d[:].rearrange("(kh kl) n -> kl kh n", kl=128))
        ko = wpool.tile([NCC, 4 * H], bf16, tag="ko")
        dma.dma_start(out=ko, in_=ko_d[:])
        oh = wpool.tile([NCC, S, BS], bf16, tag="oh")
        dma.dma_start(out=oh, in_=oh_d[:])
        wg = wpool.tile([128, 4, NCC], bf16, tag="wg")
        dma.dma_start(out=wg, in_=wg_d[:].rearrange("(hh hl) n -> hl hh n", hl=128))
        bg = wpool.tile([BS, NCC], f32, tag="bg")
        dma.dma_start(out=bg, in_=bg_d[:])
        eye = wpool.tile([64, 64], bf16, tag="eye")
        dma.dma_start(out=eye, in_=eye_d[:])
        for g in range(NG):
            dma.dma_start(out=bHc[g], in_=bHc_d[g].rearrange("k p c -> p k c"))
        ones1 = wpool.tile([128, 1], bf16, tag="ones1")
        nc.vector.memset(ones1, 1.0)

        # affine block-diag holders (zeroed once; only diagonal slots written)
        ablk = [wpool.tile([128, ABW], bf16, tag=f"ablk{g}", name=f"ablk{g}")
                for g in range(NG)]
        for g in range(NG):
            nc.vector.memset(ablk[g], 0.0)

        def ablk_lhsT(g, kt):
            ab = ablk[g][:]
            return _b.AP(tensor=ab.tensor, offset=ab.offset + ABOFF - 31 * kt,
                         ap=[ab.ap[0], [16, 32]])

        # initial state (joint across groups)
        hTj = [state.tile([128, 4, BS], bf16, tag="hT", name="hT0")]
        nc.vector.memset(hTj[0], 0.0)
        c_stj = [state.tile([BS, H], f32, tag="c", name="c0")]
        nc.vector.memset(c_stj[0], 0.0)

        # ---- prolog: HprojT[g] = (batch_H @ Wi)^T + bh ----
        hprojT = [big.tile([128, 4, T * GB], bf16, tag=f"hp{g}", name=f"hp{g}")
                  for g in range(NG)]
        for g in range(NG):
            for m in range(4):
                for n in range(NCH):
                    ps = pbig.tile([128, 512], f32, tag="pbig")
                    for k in range(4):
                        nc.tensor.matmul(
                            ps,
                            wi[:, k, m * 128:(m + 1) * 128],
                            bHT[g][:, k, n * 512:(n + 1) * 512],
                            start=(k == 0), stop=(k == 3),
                        )
                    nc.scalar.activation(
                        out=hprojT[g][:, m, n * 512:(n + 1) * 512], in_=ps,
                        func=AF.Identity, bias=bh[:, m:m + 1], scale=1.0,
                    )

        def bcast_q(qTb, g, k):
            # qTb[:, k, g, :] = [128, 32] in B' order -> [128, T(bcast), 32]
            ap2 = qTb[:, k, g, 0, :]
            return _b.AP(tensor=ap2.tensor, offset=ap2.offset,
                         ap=[ap2.ap[0], [0, T], [1, 32]])

        # z columns reordered to (i, f, o, g) so the three sigmoid gates are
        # contiguous and one ACT instruction covers them
        gate_sl = {"i": 0, "f": 1, "o": 2, "g": 3}
        GORDER = ("i", "f", "o", "g")

        # pqd bank layout (one PSUM bank, f32 cols): [0,256) = qT (4 m-chunks
        # of 64 b-cols), [256,258) = softmax denominators, [258,354) = wgen
        def phase_qg(s, g, pqd, qTb):
            # q for group g of the NEXT step, from this step's hT_g
            for m in range(4):
                for k in range(4):
                    nc.tensor.matmul(
                        pqd[:, m * 64 + 32 * g:m * 64 + 32 * g + 32],
                        wh[:, k, m * 128:(m + 1) * 128],
                        hTj[0][:, k, 32 * g:32 * g + 32],
                        start=(k == 0), stop=(k == 3))
            # SBUF copy with columns permuted to B' = 16*(b%2) + b//2 order
            # so the broadcast-add APs stay stride-1 packed (DVE 2x mode)
            pqa = pqd[:]
            src = _b.AP(tensor=pqa.tensor, offset=pqa.offset + 32 * g,
                        ap=[pqa.ap[0], [64, 4], [1, 2], [2, 16]])
            nc.vector.tensor_copy(qTb[:, :, g, :, :], src)

        def phase_zh(s, pz):
            for gname in GORDER:
                zsl = slice(gate_sl[gname] * 512, (gate_sl[gname] + 1) * 512)
                for k in range(4):
                    nc.tensor.matmul(pz[:, zsl], hTj[0][:, k, :], rr[:, k, zsl],
                                     start=(k == 0), stop=False)
                nc.tensor.matmul(pz[:, zsl], oh[:, s, :], ko[:, zsl],
                                 start=False, stop=False)

        def att_front(s, g, k, qTb, th):
            thv = th[:, k, :].rearrange("p (t b) -> p t b", t=T)
            hpv = hprojT[g][:, k, :].rearrange("p (t b) -> p t b", t=T)
            nc.vector.tensor_add(thv, hpv, bcast_q(qTb, g, k))
            nc.scalar.activation(out=th[:, k, :], in_=th[:, k, :], func=AF.Tanh)

        def att_emm(s, g, k, th, pe):
            for j in range(NCH):
                bp = 32 * j
                nc.tensor.matmul(pe[bp:bp + 32, :], ws[:, k, :],
                                 th[:, k, j * 512:(j + 1) * 512],
                                 start=(k == 0), stop=(k == 3),
                                 tile_position=(0, bp))

        def att_tail(s, g, pe):
            # exp + contiguous scatter into the affine block-diag
            expe = small.tile([128, 512], bf16, tag=f"expe{g}", bufs=1,
                              name=f"expe{g}_{s}")
            nc.scalar.activation(out=expe, in_=pe, func=AF.Exp)
            ea = expe[:]
            ab = ablk[g][:]
            ps_e = ea.ap[0][0]
            ps_a = ab.ap[0][0]
            for par in (0, 1):
                src = _b.AP(tensor=ea.tensor, offset=ea.offset + 16 * par,
                            ap=[[32 * ps_e, 4], [32, 16], [1, 16]])
                dst = _b.AP(tensor=ab.tensor,
                            offset=ab.offset + 64 * par * ps_a + ABOFF + 16 * par,
                            ap=[[ps_a, 64], [1, 16]])
                dma.dma_start(out=dst, in_=src)

        def phase_ctx_g(s, g, pqd, pctx, rcp):
            # ctx at col group 32g, denominator at col group 64+32g
            for kt in range(GB // 2):
                lt = ablk_lhsT(g, kt)
                nc.tensor.matmul(pctx[32 * g:32 * g + 32, :], lt,
                                 bHc[g][:, kt, :],
                                 start=(kt == 0), stop=(kt == GB // 2 - 1))
                nc.tensor.matmul(pqd[64 + 32 * g:96 + 32 * g, 256 + g:257 + g],
                                 lt, ones1,
                                 start=(kt == 0), stop=(kt == GB // 2 - 1),
                                 tile_position=(0, 64 + 32 * g))
            gsl = slice(64 + 32 * g, 96 + 32 * g)
            nc.vector.reciprocal(rcp[gsl, g:g + 1], pqd[gsl, 256 + g:257 + g])

        def phase_scale_x(s, g, pctx, rcp, xTc):
            gsl = slice(g * GB, (g + 1) * GB)
            ctx_g = small.tile([GB, C], bf16, tag=f"ctxg{g}", bufs=2,
                               name=f"ctx{g}_{s}")
            nc.vector.tensor_scalar_mul(ctx_g,
                                        pctx[32 * g:32 * g + 32, :],
                                        rcp[64 + 32 * g:96 + 32 * g, g:g + 1])
            xTp = psm.tile([128, 4, GB], bf16, tag="psm", name=f"xTp{g}_{s}")
            for k in range(4):
                nc.tensor.transpose(xTp[:, k, :],
                                    ctx_g[:, k * 128:(k + 1) * 128],
                                    eye[0:GB, 0:GB])
            nc.vector.tensor_copy(xTc[:, :, gsl], xTp)

        def phase_kc(s, g, xTc, pz):
            gsl = slice(g * GB, (g + 1) * GB)
            for gname in GORDER:
                zsl = slice(gate_sl[gname] * 512, (gate_sl[gname] + 1) * 512)
                for k in range(4):
                    nc.tensor.matmul(pz[gsl, zsl],
                                     xTc[:, k, gsl], kc[:, k, zsl],
                                     start=False, stop=(k == 3))

        def phase_gates(s, g, pz, c_new, h_bf):
            # per-gate ACTs so the c-chain pipelines behind f/i/g
            # (all tensor_tensor operands sliced to the same partition base)
            gsl = slice(g * GB, (g + 1) * GB)
            sig = {}
            for gname in ("f", "i", "g", "o"):
                zsl = slice(gate_sl[gname] * 512, (gate_sl[gname] + 1) * 512)
                g_sb = gates.tile([BS, 512], f32, tag=f"gate{g}", bufs=3)
                if gname == "g":
                    nc.scalar.activation(out=g_sb[gsl, :], in_=pz[gsl, zsl],
                                         func=AF.Tanh)
                else:
                    nc.scalar.activation(out=g_sb[gsl, :], in_=pz[gsl, zsl],
                                         func=AF.Tanh, scale=0.5)
                    nc.vector.tensor_scalar(out=g_sb[gsl, :], in0=g_sb[gsl, :],
                                            scalar1=0.5, scalar2=0.5,
                                            op0=mybir.AluOpType.mult,
                                            op1=mybir.AluOpType.add)
                sig[gname] = g_sb
                if gname == "i":
                    t1 = gates.tile([BS, H], f32, tag=f"tmp{g}", bufs=2,
                                    name=f"t1_{g}_{s}")
                    sig["t1"] = t1
                    nc.vector.tensor_mul(t1[gsl, :], sig["f"][gsl, :],
                                         c_stj[0][gsl, :])
                elif gname == "g":
                    t2 = gates.tile([BS, H], f32, tag=f"tmp{g}", bufs=2)
                    nc.vector.tensor_mul(t2[gsl, :], sig["i"][gsl, :],
                                         sig["g"][gsl, :])
                    nc.vector.tensor_add(c_new[gsl, :], sig["t1"][gsl, :],
                                         t2[gsl, :])
            tc_sb = gates.tile([BS, H], f32, tag=f"tmp{g}", bufs=2)
            nc.scalar.activation(out=tc_sb[gsl, :], in_=c_new[gsl, :],
                                 func=AF.Tanh)
            nc.vector.tensor_mul(h_bf[gsl, :], sig["o"][gsl, :], tc_sb[gsl, :])

        def phase_hq_g(s, g, h_bf, hTn, pqd, qTb):
            # per-group hT via PE transpose + next-step q_g: group 0 runs right
            # after gates0 so next step's attention front-runs the g1 tail
            gsl = slice(g * GB, (g + 1) * GB)
            hTp = psm.tile([128, 4, GB], bf16, tag="psm", name=f"hTp{g}_{s}")
            for k in range(4):
                # eye's diagonal block at rows 32g.. matches h_bf's base
                nc.tensor.transpose(hTp[:, k, :],
                                    h_bf[gsl, k * 128:(k + 1) * 128],
                                    eye[gsl, gsl])
            nc.vector.tensor_copy(hTn[:, :, gsl], hTp)
            for m in range(4):
                for k in range(4):
                    nc.tensor.matmul(
                        pqd[:, m * 64 + 32 * g:m * 64 + 32 * g + 32],
                        wh[:, k, m * 128:(m + 1) * 128],
                        hTn[:, k, gsl],
                        start=(k == 0), stop=(k == 3))
            pqa = pqd[:]
            src = _b.AP(tensor=pqa.tensor, offset=pqa.offset + 32 * g,
                        ap=[pqa.ap[0], [64, 4], [1, 2], [2, 16]])
            nc.vector.tensor_copy(qTb[:, :, g, :, :], src)
            # per-group wgen + bias + output: g0's leaves the critical tail
            for k in range(4):
                nc.tensor.matmul(pqd[32 * g:32 * g + 32, 258:258 + NCC],
                                 hTn[:, k, gsl], wg[:, k, :],
                                 start=(k == 0), stop=(k == 3))
            pr_sb = small.tile([BS, NCC], f32, tag="pr_sb", bufs=2,
                               name=f"pr{g}_{s}")
            nc.vector.tensor_add(pr_sb[gsl, :],
                                 pqd[32 * g:32 * g + 32, 258:258 + NCC],
                                 bg[gsl, :])
            dma.dma_start(out=out_d[gsl, s, :], in_=pr_sb[gsl, :])

        def phase_tail2(s, h_bf, hTn, pqd, qTb):
            phase_hq_g(s, 1, h_bf, hTn, pqd, qTb)
            hTj[0] = hTn

        # software-pipelined main loop: group 0's next-step q/attention overlap
        # group 1's current-step tail via per-group q/hT/wgen splits
        qTb = small.tile([128, 4, NG, 2, 16], bf16, tag="qT", bufs=2, name="qT0")
        nc.vector.memset(qTb, 0.0)
        pz = pzg.tile([BS, 4 * 512], f32, tag="pz", name="pz_0")
        phase_zh(0, pz)
        for s in range(S):
            th = [None, None]
            th[0] = big.tile([128, 4, T * GB], bf16, tag="th0", name=f"th0_{s}")
            pe0 = pbig.tile([128, 512], f32, tag="pbig", name=f"pe0_{s}")
            for k in range(4):
                att_front(s, 0, k, qTb, th[0])
                att_emm(s, 0, k, th[0], pe0)
            att_tail(s, 0, pe0)
            th[1] = big.tile([128, 4, T * GB], bf16, tag="th1", name=f"th1_{s}")
            for k in range(4):
                att_front(s, 1, k, qTb, th[1])
            pe1 = pbig.tile([128, 512], f32, tag="pbig", name=f"pe1_{s}")
            for k in range(4):
                att_emm(s, 1, k, th[1], pe1)
            pqd = pqdp.tile([128, 354], f32, tag="pqd", name=f"pqd_{s}")
            rcp = tiny.tile([128, 2], f32, tag="rcp", name=f"rcp_{s}")
            xTc = small.tile([128, 4, BS], bf16, tag="xTc", bufs=2, name=f"xTc_{s}")
            pctx0 = pbig.tile([128, 512], f32, tag="pbig", name=f"pctx0_{s}")
            phase_ctx_g(s, 0, pqd, pctx0, rcp)
            phase_scale_x(s, 0, pctx0, rcp, xTc)
            phase_kc(s, 0, xTc, pz)
            att_tail(s, 1, pe1)
            c_new = state.tile([BS, H], f32, tag="c", name=f"c_{s}")
            h_bf = small.tile([BS, H], bf16, tag="h_bf", bufs=1, name=f"h_bf_{s}")
            qTb_n = small.tile([128, 4, NG, 2, 16], bf16, tag="qT", bufs=2,
                               name=f"qT_{s + 1}")
            hTn = small.tile([128, 4, BS], bf16, tag="hT", bufs=2,
                             name=f"hT_{s}")
            phase_gates(s, 0, pz, c_new, h_bf)
            phase_hq_g(s, 0, h_bf, hTn, pqd, qTb_n)
            pctx1 = pbig.tile([128, 512], f32, tag="pbig", name=f"pctx1_{s}")
            phase_ctx_g(s, 1, pqd, pctx1, rcp)
            phase_scale_x(s, 1, pctx1, rcp, xTc)
            phase_kc(s, 1, xTc, pz)
            phase_gates(s, 1, pz, c_new, h_bf)
            c_stj[0] = c_new
            phase_tail2(s, h_bf, hTn, pqd, qTb_n)
            qTb = qTb_n
            if s + 1 < S:
                pz = pzg.tile([BS, 4 * 512], f32, tag="pz", name=f"pz_{s + 1}")
                phase_zh(s + 1, pz)

    nc.finalize()
    return nc


def _prep_core(inputs, i):
    bsl = slice(i * BS, (i + 1) * BS)
    bh_i = np.asarray(inputs["batch_H"][bsl], np.float32)          # [64, 64, 512]
    text_i = np.asarray(inputs["text"][bsl])                       # [64, 26]
    bh_g = bh_i.reshape(NG, GB, T, C)
    m = {}
    # bHT columns in B' = 16*(b%2) + b//2 order (for the contiguous scatter)
    perm = np.array([2 * (bp % 16) + bp // 16 for bp in range(GB)])
    bht = np.ascontiguousarray(bh_g.transpose(0, 3, 2, 1))         # [g, c, t, b]
    m["bHT"] = np.ascontiguousarray(bht[:, :, :, perm]).astype(BF)
    m["bHc"] = np.ascontiguousarray(bh_g.reshape(NG, GB // 2, 128, C)).astype(BF)
    m["wi"] = np.asarray(inputs["Wi"], np.float32).astype(BF)
    m["wh"] = np.asarray(inputs["Wh"], np.float32).astype(BF)
    m["bh"] = np.ascontiguousarray(
        np.asarray(inputs["bh"], np.float32).reshape(4, 128).T)
    wsr = np.ascontiguousarray(
        np.asarray(inputs["Ws"], np.float32)[:, 0].reshape(4, 128).T).astype(BF)
    m["ws"] = np.repeat(wsr[:, :, None], 32, axis=2)
    lk = np.asarray(inputs["lstm_kernel"], np.float32)
    lb = np.asarray(inputs["lstm_bias"], np.float32)
    # z blocks reordered (i, f, g, o) -> (i, f, o, g)
    zperm = np.concatenate([np.arange(0, 1024), np.arange(1536, 2048),
                            np.arange(1024, 1536)])
    m["kc"] = lk[:C][:, zperm].astype(BF)
    m["ko"] = (lk[C:] + lb[None, :])[:, zperm].astype(BF)
    m["rr"] = np.asarray(inputs["lstm_rec"], np.float32)[:, zperm].astype(BF)
    m["oh"] = (np.arange(NCC)[:, None, None] == text_i.T[None, :, :]).astype(BF)
    m["wg"] = np.asarray(inputs["Wgen"], np.float32).astype(BF)
    m["bg"] = np.tile(np.asarray(inputs["bgen"], np.float32)[None, :], (BS, 1))
    m["eye"] = np.eye(64, dtype=np.float32).astype(BF)
    return m


def kernel(_trace=False, **inputs):
    from concourse import bass_utils
    if "nc" not in _CACHE:
        _CACHE["nc"] = build_bass()
    nc = _CACHE["nc"]
    in_maps = [_prep_core(inputs, i) for i in range(NCORES)]
    res = bass_utils.run_bass_kernel_spmd(nc, in_maps, list(range(NCORES)),
                                          trace=_trace)
    _CACHE["last_result"] = res
    out = np.concatenate([r["out"] for r in res.results], axis=0)
    return out.astype(np.float32)
